# revision 8
# baseline (speedup 1.0000x reference)
"""Multi-head attention (B=2, S=2048, D=1024, H=16) on 8 trn2 NeuronCores.

Sharding: core c -> batch b = c//4, head-group g = c%4 (4 heads each).
Tensor-parallel on heads: each core projects q/k/v for its 4 heads only
(column-sharded W_q/W_k/W_v), runs full-sequence attention for those heads,
all-gathers the per-head attention outputs within its 4-core batch group,
and computes a 256-column slice of the final W_o projection. The host
reassembles the [2, 2048, 1024] output from the 8 per-core [256, 2048]
(transposed) slices.

All matmuls run in float32r (single-pass fp32 on the PE, ~1e-4 rel err).
Layouts are chosen so no on-device transposes are needed: the host supplies
X^T and W^T shards, projections produce q^T/k^T ([d, s]) and natural v
([s, d]); scores are computed transposed ([s_k, s_q]) so softmax sums come
free as an extra ones-column in the PV matmul, and exp() is fused into the
PSUM->SBUF eviction on the scalar engine.
"""

import numpy as np

import concourse.bacc as bacc
import concourse.mybir as mybir
import concourse.tile as tile
from concourse import bass_utils

B, S, D, H = 2, 2048, 1024, 16
Dh = D // H  # 64
N_CORES = 8
HL = H // 4  # heads per core (4)
DL = HL * Dh  # local head dims (256)
P = 128
E_TILES = D // P  # 8
KT = S // P  # 16 key tiles
QB = 512  # query block (moving free dim)
NQB = S // QB  # 4

f32 = mybir.dt.float32
f32r = mybir.dt.float32r
AF = mybir.ActivationFunctionType

TRACE = False  # test harness sets kernel.TRACE = True for profiling


def _build():
    nc = bacc.Bacc("TRN2", target_bir_lowering=False, debug=False,
                   num_devices=N_CORES)

    XqT = nc.dram_tensor("XqT", [D, S], f32, kind="ExternalInput").ap()
    XkT = nc.dram_tensor("XkT", [D, S], f32, kind="ExternalInput").ap()
    XvT = nc.dram_tensor("XvT", [D, S], f32, kind="ExternalInput").ap()
    WqT = nc.dram_tensor("WqT", [D, DL], f32, kind="ExternalInput").ap()
    WkT = nc.dram_tensor("WkT", [D, DL], f32, kind="ExternalInput").ap()
    WvT = nc.dram_tensor("WvT", [D, DL], f32, kind="ExternalInput").ap()
    WoT = nc.dram_tensor("WoT", [D, DL], f32, kind="ExternalInput").ap()
    bq = nc.dram_tensor("bq", [DL], f32, kind="ExternalInput").ap()
    bk = nc.dram_tensor("bk", [DL], f32, kind="ExternalInput").ap()
    bv = nc.dram_tensor("bv", [DL], f32, kind="ExternalInput").ap()
    bo = nc.dram_tensor("bo", [DL], f32, kind="ExternalInput").ap()
    out = nc.dram_tensor("out", [DL, S], f32, kind="ExternalOutput").ap()

    with tile.TileContext(nc) as tc:
        with (
            tc.tile_pool(name="const", bufs=1) as cp,
            tc.tile_pool(name="xs", bufs=2) as xsp,
            tc.tile_pool(name="xvs", bufs=3) as xvsp,
            tc.tile_pool(name="qk", bufs=1) as qkp,
            tc.tile_pool(name="vt", bufs=1) as vtp,
            tc.tile_pool(name="exp", bufs=2) as expp,
            tc.tile_pool(name="nrm", bufs=1) as nrmp,
            tc.tile_pool(name="ao", bufs=1) as aop,
            tc.tile_pool(name="res", bufs=1) as resp,
            tc.tile_pool(name="dram", bufs=1, space="DRAM") as dram,
        ):
            # ---- constants / weights ----
            wq = cp.tile([P, E_TILES, DL], f32r, tag="wq")
            wk = cp.tile([P, E_TILES, DL], f32r, tag="wk")
            wv = cp.tile([P, E_TILES, DL], f32r, tag="wv")
            wo = cp.tile([P, E_TILES, DL], f32r, tag="wo")
            for w_sb, w_dr in ((wq, WqT), (wk, WkT), (wv, WvT), (wo, WoT)):
                nc.sync.dma_start(
                    w_sb[:],
                    w_dr.rearrange("(t p) n -> p t n", p=P).bitcast(f32r),
                )

            bq_c = cp.tile([P, 2], f32, tag="bq")
            bk_c = cp.tile([P, 2], f32, tag="bk")
            bo_c = cp.tile([P, 2], f32, tag="bo")
            for b_sb, b_dr in ((bq_c, bq), (bk_c, bk), (bo_c, bo)):
                nc.sync.dma_start(b_sb[:], b_dr.rearrange("(t p) -> p t", p=P))
            bq8 = cp.tile([P, 2], f32, tag="bq8")
            nc.scalar.mul(bq8[:], bq_c[:], 0.125)

            bv_row = cp.tile([1, DL], f32r, tag="bvr")
            nc.sync.dma_start(bv_row[:], bv[None, :].bitcast(f32r))
            ones_f = cp.tile([1, P], f32, tag="onesf")
            nc.gpsimd.memset(ones_f[:], 1.0)
            ones_col = cp.tile([1, P], f32r, tag="ones")
            nc.vector.tensor_copy(ones_col[:], ones_f[:])
            ones64 = cp.tile([P, KT * HL], f32, tag="ones64")
            nc.gpsimd.memset(ones64[:], 1.0)

            # preload the exp table while DMAs stream
            warm = cp.tile([P, 1], f32, tag="warm")
            nc.gpsimd.memset(warm[:], 0.0)
            nc.scalar.activation(warm[:], warm[:], AF.Exp)

            # ---- V projection (natural layout [s, h, d+1], +bias, ones) ----
            v_sb = vtp.tile([P, KT, HL, Dh + 1], f32r)
            nc.vector.tensor_copy(
                v_sb[:, :, :, Dh],
                ones64.rearrange("p (a b) -> p a b", a=KT),
            )
            with tc.tile_pool(name="psv", bufs=1, space="PSUM") as psvp:
                for sb in range(KT):
                    xvs = xvsp.tile([P, E_TILES, P], f32r, tag="xvs")
                    nc.sync.dma_start(
                        xvs[:],
                        XvT[:, sb * P : (sb + 1) * P]
                        .rearrange("(t p) s -> p t s", p=P)
                        .bitcast(f32r),
                    )
                    psv = psvp.tile([P, DL], f32, tag="pv", bufs=4)
                    for e in range(E_TILES):
                        nc.tensor.matmul(
                            psv[:],
                            xvs[:, e, :],
                            wv[:, e, :],
                            start=(e == 0),
                            stop=False,
                        )
                    nc.tensor.matmul(
                        psv[:], ones_col[:], bv_row[:], start=False, stop=True
                    )
                    nc.vector.tensor_copy(
                        v_sb[:, sb, :, 0:Dh],
                        psv.rearrange("p (h d) -> p h d", h=HL),
                    )

            # ---- Q / K projections -> qT, kT in [d, s] layout (f32r) ----
            # q is pre-scaled by 1/sqrt(Dh) = 0.125 (bias scaled to match).
            qT = [
                qkp.tile([P, S], f32r, tag=f"qT{i}", name=f"qT{i}")
                for i in range(2)
            ]
            kT = [
                qkp.tile([P, S], f32r, tag=f"kT{i}", name=f"kT{i}")
                for i in range(2)
            ]
            with tc.tile_pool(name="psq", bufs=1, space="PSUM") as psqp:
                for xdr, wt, dest, scale, bias in (
                    (XqT, wq, qT, 0.125, bq8),
                    (XkT, wk, kT, 1.0, bk_c),
                ):
                    psq = [
                        psqp.tile(
                            [P, QB], f32, tag=f"pp{i}", name=f"pp{i}", bufs=1
                        )
                        for i in range(8)
                    ]
                    for e in range(E_TILES):
                        xe = xsp.tile([P, S], f32r, tag="xs")
                        nc.sync.dma_start(
                            xe[:], xdr[e * P : (e + 1) * P, :].bitcast(f32r)
                        )
                        for dt in range(2):
                            for qb in range(NQB):
                                nc.tensor.matmul(
                                    psq[dt * NQB + qb][:],
                                    wt[:, e, dt * P : (dt + 1) * P],
                                    xe[:, qb * QB : (qb + 1) * QB],
                                    start=(e == 0),
                                    stop=(e == E_TILES - 1),
                                )
                    for dt in range(2):
                        for qb in range(NQB):
                            nc.scalar.activation(
                                dest[dt][:, qb * QB : (qb + 1) * QB],
                                psq[dt * NQB + qb][:],
                                AF.Identity,
                                bias=bias[:, dt : dt + 1],
                                scale=scale,
                            )

            # ---- attention, head by head ----
            ag_in = [
                dram.tile([P, S], f32, tag=f"agi{i}", name=f"agi{i}")
                for i in range(2)
            ]
            ag_out = [
                dram.tile([4 * P, S], f32, tag=f"ago{i}", name=f"ago{i}")
                for i in range(2)
            ]

            with (
                tc.tile_pool(name="pss", bufs=1, space="PSUM") as pssp,
                tc.tile_pool(name="pso", bufs=1, space="PSUM") as psop,
            ):
                for h in range(HL):
                    ht, hp = h // 2, 64 * (h % 2)
                    pso = psop.tile([Dh + 1, S], f32, tag="pso")
                    for kt in range(KT):
                        pss = pssp.tile([P, S], f32, tag="pss")
                        for qb in range(NQB):
                            nc.tensor.matmul(
                                pss[:, qb * QB : (qb + 1) * QB],
                                kT[ht][hp : hp + Dh, kt * P : (kt + 1) * P],
                                qT[ht][hp : hp + Dh, qb * QB : (qb + 1) * QB],
                                start=True,
                                stop=True,
                            )
                        ex = expp.tile([P, S], f32r, tag="ex")
                        nc.scalar.activation(ex[:], pss[:], AF.Exp)
                        for qb in range(NQB):
                            nc.tensor.matmul(
                                pso[:, qb * QB : (qb + 1) * QB],
                                v_sb[:, kt, h, :],
                                ex[:, qb * QB : (qb + 1) * QB],
                                start=(kt == 0),
                                stop=(kt == KT - 1),
                            )
                    sums64 = nrmp.tile([Dh + 1, S], f32, tag="sm64")
                    nc.vector.tensor_copy(
                        sums64[Dh : Dh + 1, :], pso[Dh : Dh + 1, :]
                    )
                    sums = nrmp.tile([1, S], f32, tag="sm")
                    nc.sync.dma_start(sums[:], sums64[Dh : Dh + 1, :])
                    recip = nrmp.tile([1, S], f32, tag="rc")
                    scratch = nrmp.tile([1, S], f32, tag="rs")
                    nc.vector.reciprocal_approx_accurate(
                        recip[:], sums[:], scratch[:]
                    )
                    bc = nrmp.tile([64, S], f32, tag="bc")
                    nc.gpsimd.partition_broadcast(bc[:], recip[:])
                    ao = aop.tile([Dh, S], f32, tag="ao", bufs=2)
                    nc.vector.tensor_mul(ao[:], pso[0:Dh, :], bc[:])
                    nc.sync.dma_start(ag_in[ht][hp : hp + Dh, :], ao[:])
                    if h % 2 == 1:
                        nc.gpsimd.collective_compute(
                            "AllGather",
                            mybir.AluOpType.bypass,
                            replica_groups=[[0, 1, 2, 3], [4, 5, 6, 7]],
                            ins=[ag_in[ht].opt()],
                            outs=[ag_out[ht].opt()],
                        )

            # ---- O projection: out.T[dl, s] = WoT.T @ attn_full.T ----
            # e-tile order: (head-pair tile ht, rank r) -> ag_out[ht] block r.
            # Host permutes WoT rows to match (see kernel()).
            with tc.tile_pool(name="pso2", bufs=1, space="PSUM") as pso2p:
                pso2 = [
                    pso2p.tile([P, QB], f32, tag=f"po{i}", name=f"po{i}", bufs=1)
                    for i in range(8)
                ]
                ei = 0
                for ht in range(2):
                    for r in range(4):
                        rhs = xsp.tile([P, S], f32r, tag="xs")
                        nc.sync.dma_start(
                            rhs[:],
                            ag_out[ht][r * P : (r + 1) * P, :].bitcast(f32r),
                        )
                        for dt in range(2):
                            for qb in range(NQB):
                                nc.tensor.matmul(
                                    pso2[dt * NQB + qb][:],
                                    wo[:, ei, dt * P : (dt + 1) * P],
                                    rhs[:, qb * QB : (qb + 1) * QB],
                                    start=(ei == 0),
                                    stop=(ei == E_TILES - 1),
                                )
                        ei += 1
                for dt in range(2):
                    ot = resp.tile([P, S], f32, tag="ot")
                    for qb in range(NQB):
                        nc.scalar.activation(
                            ot[:, qb * QB : (qb + 1) * QB],
                            pso2[dt * NQB + qb][:],
                            AF.Identity,
                            bias=bo_c[:, dt : dt + 1],
                            scale=1.0,
                        )
                    nc.sync.dma_start(out[dt * P : (dt + 1) * P, :], ot[:])

    nc.compile()
    return nc


def kernel(**inputs):
    Q = np.asarray(inputs["Q"], dtype=np.float32)
    K = np.asarray(inputs["K"], dtype=np.float32)
    V = np.asarray(inputs["V"], dtype=np.float32)
    Wq = np.asarray(inputs["Wq"], dtype=np.float32)
    Wk = np.asarray(inputs["Wk"], dtype=np.float32)
    Wv = np.asarray(inputs["Wv"], dtype=np.float32)
    Wo = np.asarray(inputs["Wo"], dtype=np.float32)
    bq = np.asarray(inputs["bq"], dtype=np.float32)
    bk = np.asarray(inputs["bk"], dtype=np.float32)
    bv = np.asarray(inputs["bv"], dtype=np.float32)
    bo = np.asarray(inputs["bo"], dtype=np.float32)

    nc = _build()

    XT = {
        b: {
            "XqT": np.ascontiguousarray(Q[b].T),
            "XkT": np.ascontiguousarray(K[b].T),
            "XvT": np.ascontiguousarray(V[b].T),
        }
        for b in range(B)
    }
    # O-projection e-tile order is (head-pair tile ht, rank r): the 128 rows
    # of attn_full^T arriving in ag_out[ht] block r are heads (4r + 2*ht),
    # (4r + 2*ht + 1), i.e. global dims [256r + 128*ht, +128).
    perm = np.concatenate(
        [
            np.arange(256 * r + 128 * ht, 256 * r + 128 * ht + 128)
            for ht in range(2)
            for r in range(4)
        ]
    )
    Wslices = {}
    for g in range(4):
        rows = slice(DL * g, DL * (g + 1))
        Wslices[g] = {
            "WqT": np.ascontiguousarray(Wq[rows].T),
            "WkT": np.ascontiguousarray(Wk[rows].T),
            "WvT": np.ascontiguousarray(Wv[rows].T),
            "WoT": np.ascontiguousarray(Wo[rows].T[perm]),
            "bq": bq[rows].copy(),
            "bk": bk[rows].copy(),
            "bv": bv[rows].copy(),
            "bo": bo[rows].copy(),
        }

    in_maps = []
    for c in range(N_CORES):
        b, g = c // 4, c % 4
        m = dict(XT[b])
        m.update(Wslices[g])
        in_maps.append(m)

    res = bass_utils.run_bass_kernel_spmd(
        nc, in_maps, core_ids=list(range(N_CORES)), trace=TRACE
    )

    full = np.empty((B, S, D), dtype=np.float32)
    for c in range(N_CORES):
        b, g = c // 4, c % 4
        full[b, :, DL * g : DL * (g + 1)] = res.results[c]["out"].T
    if TRACE:
        kernel.last_result = res
    return full


# revision 10
# speedup vs baseline: 1.3796x; 1.3796x over previous
"""Multi-head attention (B=2, S=2048, D=1024, H=16) on 8 trn2 NeuronCores.

Sharding: core c -> batch b = c//4, head-group g = c%4 (4 heads each).
Tensor-parallel on heads: each core projects q/k/v for its 4 heads only
(column-sharded W_q/W_k/W_v), runs full-sequence attention for those heads,
all-gathers the per-head attention outputs within its 4-core batch group,
and computes a 256-column slice of the final W_o projection. The host
reassembles the [2, 2048, 1024] output from the 8 per-core [256, 2048]
(transposed) slices.

All matmuls run in float32r (single-pass fp32 on the PE, ~1e-4 rel err).
The host supplies X^T and W^T shards so projections need no on-device
transposes: q^T/k^T/v^T come out in [d, s] layout; v^T is then PE-transposed
into natural [s, d] tiles for the PV matmul. Scores are computed transposed
([s_k, s_q]) so softmax sums come free as an extra ones-column in the PV
matmul, and exp() is fused into the PSUM->SBUF eviction on the scalar
engine. Scores/exp/PV are pipelined over double-buffered half-width PSUM
tiles; the softmax normalization (reciprocal + partition-broadcast +
multiply) overlaps the next head's attention. The all-gather is split
2/1/1 heads so most of it hides under attention.
"""

import numpy as np

import concourse.bacc as bacc
import concourse.mybir as mybir
import concourse.tile as tile
from concourse import bass_utils
from concourse.masks import make_identity

B, S, D, H = 2, 2048, 1024, 16
Dh = D // H  # 64
N_CORES = 8
HL = H // 4  # heads per core (4)
DL = HL * Dh  # local head dims (256)
P = 128
E_TILES = D // P  # 8
KT = S // P  # 16 key tiles
QB = 512  # matmul moving block
SH = S // 2  # attention s_q half (1024)

f32 = mybir.dt.float32
f32r = mybir.dt.float32r
AF = mybir.ActivationFunctionType

TRACE = False  # test harness sets kernel.TRACE = True for profiling


def _build():
    nc = bacc.Bacc("TRN2", target_bir_lowering=False, debug=False,
                   num_devices=N_CORES)

    XqT = nc.dram_tensor("XqT", [D, S], f32, kind="ExternalInput").ap()
    XkT = nc.dram_tensor("XkT", [D, S], f32, kind="ExternalInput").ap()
    XvT = nc.dram_tensor("XvT", [D, S], f32, kind="ExternalInput").ap()
    WqT = nc.dram_tensor("WqT", [D, DL], f32, kind="ExternalInput").ap()
    WkT = nc.dram_tensor("WkT", [D, DL], f32, kind="ExternalInput").ap()
    WvT = nc.dram_tensor("WvT", [D, DL], f32, kind="ExternalInput").ap()
    WoT = nc.dram_tensor("WoT", [D, DL], f32, kind="ExternalInput").ap()
    bq = nc.dram_tensor("bq", [DL], f32, kind="ExternalInput").ap()
    bk = nc.dram_tensor("bk", [DL], f32, kind="ExternalInput").ap()
    bv = nc.dram_tensor("bv", [DL], f32, kind="ExternalInput").ap()
    bo = nc.dram_tensor("bo", [DL], f32, kind="ExternalInput").ap()
    out = nc.dram_tensor("out", [DL, S], f32, kind="ExternalOutput").ap()

    with tile.TileContext(nc) as tc:
        with (
            tc.tile_pool(name="const", bufs=1) as cp,
            tc.tile_pool(name="xs", bufs=4) as xsp,
            tc.tile_pool(name="qk", bufs=1) as qkp,
            tc.tile_pool(name="vtt", bufs=1) as vttp,
            tc.tile_pool(name="vt", bufs=1) as vtp,
            tc.tile_pool(name="exp", bufs=3) as expp,
            tc.tile_pool(name="nrm", bufs=2) as nrmp,
            tc.tile_pool(name="ao", bufs=2) as aop,
            tc.tile_pool(name="res", bufs=1) as resp,
            tc.tile_pool(name="dram", bufs=1, space="DRAM") as dram,
        ):
            # ---- constants / weights ----
            wq = cp.tile([P, E_TILES, DL], f32r, tag="wq")
            wk = cp.tile([P, E_TILES, DL], f32r, tag="wk")
            wv = cp.tile([P, E_TILES, DL], f32r, tag="wv")
            wo = cp.tile([P, E_TILES, DL], f32r, tag="wo")
            for w_sb, w_dr in ((wq, WqT), (wk, WkT), (wv, WvT), (wo, WoT)):
                nc.sync.dma_start(
                    w_sb[:],
                    w_dr.rearrange("(t p) n -> p t n", p=P).bitcast(f32r),
                )

            bq_c = cp.tile([P, 2], f32, tag="bq")
            bk_c = cp.tile([P, 2], f32, tag="bk")
            bv_c = cp.tile([P, 2], f32, tag="bv")
            bo_c = cp.tile([P, 2], f32, tag="bo")
            for b_sb, b_dr in ((bq_c, bq), (bk_c, bk), (bv_c, bv), (bo_c, bo)):
                nc.sync.dma_start(b_sb[:], b_dr.rearrange("(t p) -> p t", p=P))
            bq8 = cp.tile([P, 2], f32, tag="bq8")
            nc.scalar.mul(bq8[:], bq_c[:], 0.125)

            ident_f = cp.tile([P, P], f32, tag="identf")
            make_identity(nc, ident_f[:])
            ident = cp.tile([P, P], f32r, tag="ident")
            nc.vector.tensor_copy(ident[:], ident_f[:])
            ones64 = cp.tile([P, KT * HL], f32, tag="ones64")
            nc.gpsimd.memset(ones64[:], 1.0)

            # preload the exp table while DMAs stream
            warm = cp.tile([P, 1], f32, tag="warm")
            nc.gpsimd.memset(warm[:], 0.0)
            nc.scalar.activation(warm[:], warm[:], AF.Exp)

            # ---- Q / K / V projections -> [d, s] layout (f32r) ----
            # q is pre-scaled by 1/sqrt(Dh) = 0.125 (bias scaled to match).
            qT = [
                qkp.tile([P, S], f32r, tag=f"qT{i}", name=f"qT{i}")
                for i in range(2)
            ]
            kT = [
                qkp.tile([P, S], f32r, tag=f"kT{i}", name=f"kT{i}")
                for i in range(2)
            ]
            vT = [
                vttp.tile([P, S], f32r, tag=f"vT{i}", name=f"vT{i}")
                for i in range(2)
            ]
            with tc.tile_pool(name="psq", bufs=1, space="PSUM") as psqp:
                for xdr, wt, dest, scale, bias in (
                    (XqT, wq, qT, 0.125, bq8),
                    (XkT, wk, kT, 1.0, bk_c),
                    (XvT, wv, vT, 1.0, bv_c),
                ):
                    psq = [
                        psqp.tile(
                            [P, QB], f32, tag=f"pp{i}", name=f"pp{i}", bufs=1
                        )
                        for i in range(8)
                    ]
                    for e in range(E_TILES):
                        xe = xsp.tile([P, S], f32r, tag="xs")
                        nc.sync.dma_start(
                            xe[:], xdr[e * P : (e + 1) * P, :].bitcast(f32r)
                        )
                        for dt in range(2):
                            for qb in range(4):
                                nc.tensor.matmul(
                                    psq[dt * 4 + qb][:],
                                    wt[:, e, dt * P : (dt + 1) * P],
                                    xe[:, qb * QB : (qb + 1) * QB],
                                    start=(e == 0),
                                    stop=(e == E_TILES - 1),
                                )
                    for dt in range(2):
                        for qb in range(4):
                            nc.scalar.activation(
                                dest[dt][:, qb * QB : (qb + 1) * QB],
                                psq[dt * 4 + qb][:],
                                AF.Identity,
                                bias=bias[:, dt : dt + 1],
                                scale=scale,
                            )

            # ---- transpose v^T -> natural v [s, h, d(+ones)] ----
            v_sb = vtp.tile([P, KT, HL, Dh + 1], f32r)
            nc.vector.tensor_copy(
                v_sb[:, :, :, Dh],
                ones64.rearrange("p (a b) -> p a b", a=KT),
            )
            with tc.tile_pool(name="pst", bufs=2, space="PSUM") as pstp:
                for dt in range(2):
                    for kt in range(KT):
                        pt = pstp.tile([P, P], f32r, tag="pt")
                        nc.tensor.transpose(
                            pt[:], vT[dt][:, kt * P : (kt + 1) * P], ident[:]
                        )
                        nc.vector.tensor_copy(
                            v_sb[:, kt, 2 * dt : 2 * dt + 2, 0:Dh],
                            pt.rearrange("s (h d) -> s h d", h=2),
                        )

            # ---- attention: heads x s_q-halves x key-tiles, pipelined ----
            ag_in = [
                dram.tile([P, S], f32, tag="agi0", name="agi0"),
                dram.tile([Dh, S], f32, tag="agi1", name="agi1"),
                dram.tile([Dh, S], f32, tag="agi2", name="agi2"),
            ]
            ag_out = [
                dram.tile([4 * P, S], f32, tag="ago0", name="ago0"),
                dram.tile([4 * Dh, S], f32, tag="ago1", name="ago1"),
                dram.tile([4 * Dh, S], f32, tag="ago2", name="ago2"),
            ]

            with (
                tc.tile_pool(name="pss", bufs=2, space="PSUM") as pssp,
                tc.tile_pool(name="pso", bufs=2, space="PSUM") as psop,
            ):
                for h in range(HL):
                    ht, hp = h // 2, 64 * (h % 2)
                    for qh in range(2):
                        pso = psop.tile([Dh + 1, SH], f32, tag="pso")
                        for kt in range(KT):
                            pss = pssp.tile([P, SH], f32, tag="pss")
                            for qb in range(2):
                                nc.tensor.matmul(
                                    pss[:, qb * QB : (qb + 1) * QB],
                                    kT[ht][
                                        hp : hp + Dh, kt * P : (kt + 1) * P
                                    ],
                                    qT[ht][
                                        hp : hp + Dh,
                                        qh * SH
                                        + qb * QB : qh * SH
                                        + (qb + 1) * QB,
                                    ],
                                    start=True,
                                    stop=True,
                                )
                            ex = expp.tile([P, SH], f32r, tag="ex")
                            nc.scalar.activation(ex[:], pss[:], AF.Exp)
                            for qb in range(2):
                                nc.tensor.matmul(
                                    pso[:, qb * QB : (qb + 1) * QB],
                                    v_sb[:, kt, h, :],
                                    ex[:, qb * QB : (qb + 1) * QB],
                                    start=(kt == 0),
                                    stop=(kt == KT - 1),
                                )
                        # normalization (overlaps next head via pso bufs=2)
                        sums64 = nrmp.tile([Dh + 1, SH], f32, tag="sm64")
                        nc.vector.tensor_copy(
                            sums64[Dh : Dh + 1, :], pso[Dh : Dh + 1, :]
                        )
                        sums = nrmp.tile([1, SH], f32, tag="sm")
                        nc.sync.dma_start(sums[:], sums64[Dh : Dh + 1, :])
                        recip = nrmp.tile([1, SH], f32, tag="rc")
                        nc.vector.reciprocal_approx_fast(recip[:], sums[:])
                        bc = nrmp.tile([64, SH], f32, tag="bc")
                        nc.gpsimd.partition_broadcast(bc[:], recip[:])
                        ao = aop.tile([Dh, SH], f32, tag="ao")
                        nc.vector.tensor_mul(ao[:], pso[0:Dh, :], bc[:])
                        if h < 2:
                            nc.sync.dma_start(
                                ag_in[0][
                                    hp : hp + Dh, qh * SH : (qh + 1) * SH
                                ],
                                ao[:],
                            )
                        else:
                            nc.sync.dma_start(
                                ag_in[h - 1][:, qh * SH : (qh + 1) * SH],
                                ao[:],
                            )
                    if h != 0:
                        gi = 0 if h == 1 else h - 1
                        nc.gpsimd.collective_compute(
                            "AllGather",
                            mybir.AluOpType.bypass,
                            replica_groups=[[0, 1, 2, 3], [4, 5, 6, 7]],
                            ins=[ag_in[gi].opt()],
                            outs=[ag_out[gi].opt()],
                        )

            # ---- O projection: out.T[dl, s] = WoT.T @ attn_full.T ----
            # e-tile order follows the AG outputs; host permutes WoT rows
            # to match (see kernel()).
            e_srcs = (
                [(0, i) for i in range(4)]
                + [(1, 0), (1, 1)]
                + [(2, 0), (2, 1)]
            )
            with tc.tile_pool(name="pso2", bufs=1, space="PSUM") as pso2p:
                pso2 = [
                    pso2p.tile(
                        [P, QB], f32, tag=f"po{i}", name=f"po{i}", bufs=1
                    )
                    for i in range(8)
                ]
                for ei, (gi, blk) in enumerate(e_srcs):
                    rhs = xsp.tile([P, S], f32r, tag="xs")
                    nc.sync.dma_start(
                        rhs[:],
                        ag_out[gi][blk * P : (blk + 1) * P, :].bitcast(f32r),
                    )
                    for dt in range(2):
                        for qb in range(4):
                            nc.tensor.matmul(
                                pso2[dt * 4 + qb][:],
                                wo[:, ei, dt * P : (dt + 1) * P],
                                rhs[:, qb * QB : (qb + 1) * QB],
                                start=(ei == 0),
                                stop=(ei == E_TILES - 1),
                            )
                for dt in range(2):
                    ot = resp.tile([P, S], f32, tag="ot")
                    for qb in range(4):
                        nc.scalar.activation(
                            ot[:, qb * QB : (qb + 1) * QB],
                            pso2[dt * 4 + qb][:],
                            AF.Identity,
                            bias=bo_c[:, dt : dt + 1],
                            scale=1.0,
                        )
                    nc.sync.dma_start(out[dt * P : (dt + 1) * P, :], ot[:])

    nc.compile()
    return nc


def kernel(**inputs):
    Q = np.asarray(inputs["Q"], dtype=np.float32)
    K = np.asarray(inputs["K"], dtype=np.float32)
    V = np.asarray(inputs["V"], dtype=np.float32)
    Wq = np.asarray(inputs["Wq"], dtype=np.float32)
    Wk = np.asarray(inputs["Wk"], dtype=np.float32)
    Wv = np.asarray(inputs["Wv"], dtype=np.float32)
    Wo = np.asarray(inputs["Wo"], dtype=np.float32)
    bq = np.asarray(inputs["bq"], dtype=np.float32)
    bk = np.asarray(inputs["bk"], dtype=np.float32)
    bv = np.asarray(inputs["bv"], dtype=np.float32)
    bo = np.asarray(inputs["bo"], dtype=np.float32)

    nc = _build()

    XT = {
        b: {
            "XqT": np.ascontiguousarray(Q[b].T),
            "XkT": np.ascontiguousarray(K[b].T),
            "XvT": np.ascontiguousarray(V[b].T),
        }
        for b in range(B)
    }
    # O-projection e-tile order follows the three AG outputs:
    # AG0 = heads {0,1} rank-major, AG1 = head 2, AG2 = head 3.
    perm = []
    for r in range(4):
        for hl in (0, 1):
            perm.extend(range(64 * (4 * r + hl), 64 * (4 * r + hl) + 64))
    for hl in (2, 3):
        for r in range(4):
            perm.extend(range(64 * (4 * r + hl), 64 * (4 * r + hl) + 64))
    perm = np.array(perm)

    Wslices = {}
    for g in range(4):
        rows = slice(DL * g, DL * (g + 1))
        Wslices[g] = {
            "WqT": np.ascontiguousarray(Wq[rows].T),
            "WkT": np.ascontiguousarray(Wk[rows].T),
            "WvT": np.ascontiguousarray(Wv[rows].T),
            "WoT": np.ascontiguousarray(Wo[rows].T[perm]),
            "bq": bq[rows].copy(),
            "bk": bk[rows].copy(),
            "bv": bv[rows].copy(),
            "bo": bo[rows].copy(),
        }

    in_maps = []
    for c in range(N_CORES):
        b, g = c // 4, c % 4
        m = dict(XT[b])
        m.update(Wslices[g])
        in_maps.append(m)

    res = bass_utils.run_bass_kernel_spmd(
        nc, in_maps, core_ids=list(range(N_CORES)), trace=TRACE
    )

    full = np.empty((B, S, D), dtype=np.float32)
    for c in range(N_CORES):
        b, g = c // 4, c % 4
        full[b, :, DL * g : DL * (g + 1)] = res.results[c]["out"].T
    if TRACE:
        kernel.kernel_last = res
        kernel.last_result = res
    return full


kernel.last_result = None
